# revision 61
# baseline (speedup 1.0000x reference)
"""Trainium2 Bass kernel for BiLinearInteractionLayer.

Computes, for every field pair p=(i,j), i<j, of F=32 fields:
    y[b, p, :] = (x[b, i, :] @ W[p].T) * x[b, j, :]
x: [4096, 32, 64] f32, W: [496, 64, 64] f32 -> y: [4096, 496, 64] f32.

Sharding: data-parallel over the batch dim across 8 NeuronCores (512
rows each); the weight stack is replicated.

The kernel is HBM-bound (the 520 MB output write dominates), so all
I/O runs in bf16: inputs are rounded on the host, the output is
upcast to f32 after the gather. Measured scale-relative max error of
the bf16 pipeline is ~7e-3 (harness gate 2e-2).

Per-core algorithm (batch tile of 128 rows at a time):
  - Host pre-transposes layouts (free): the contraction dim d lands on
    SBUF partitions with clean contiguous DMAs, no on-device transposes.
  - For each first-field i, the pairs (i, i+1..31) form a contiguous
    run in both the pair axis and the transposed weight columns: one
    stationary xT_i [64d, 128b] serves bf16 matmuls streaming 512-col
    chunks of W^T into single-bank PSUM tiles (ring of 4 per parity).
    Even fields sit on PE row group 0, odd on row group 2; runs of the
    two parities alternate so stationary loads overlap the other
    stream's matmuls.
  - Epilogue (the y = proj * xj multiply) is split across engines by a
    rate-balanced per-run plan: DVE multiplies most runs straight out
    of PSUM (1x PSUM-read mode, ~106 G elem/s); for the rest, ACT
    copies each chunk PSUM->SBUF bf16 (~104 G elem/s) and GpSimd does
    one multiply per run (~63 G elem/s). DVE-1x and GpSimd use
    disjoint SBUF ports, so the three engines run concurrently (DVE
    packed 2x modes are avoided: they serialize against GpSimd on the
    shared SBUF port pair).
  - Outputs accumulate in large SBUF tiles covering up to 8192
    contiguous y columns and are flushed to HBM in rolling >=2048-col
    DMAs (16 KB row segments), overlapping compute at sub-chunk
    granularity.
  - Tile 0's x/xt DMAs are issued before the weight stack and each
    tile prefetches the next tile's inputs, so compute starts ~3 us in
    and never stalls on input loads.
"""

import itertools

import numpy as np

import concourse.bass as bass
import concourse.mybir as mybir
import concourse.tile as _tile
from concourse.bass_utils import run_bass_kernel_spmd
from concourse.tile import TileContext
from concourse.tile_scheduler import N_PROCS
from concourse.vector_clock import ScopedClock, VectorClock

# --------------------------------------------------------------------------
# Tail-drain patch: the staged walrus rejects >1 sync-wait command on a
# TPB_CTRL (Drain) instruction, but the stock Tile tail-drain attaches one
# wait per outstanding sem lane to a single Drain. Replace it with a ladder
# of single-wait SP nops (one per proc lane) followed by a wait-less drain.
# --------------------------------------------------------------------------


# Skipping the end-of-NEFF semaphore clear (~9us serial tail) HANGS the
# second execution of the same NEFF — the runtime does NOT re-zero
# semaphores between executions. Must stay False.
_SKIP_SEM_CLEAR = [False]


def _split_drain_and_barrier(self, tick_clock, wait_clock):
    nc = self.nc
    g = tick_clock.global_clock
    for p in range(N_PROCS):
        tick = g.peek_next(p) - 1
        if tick <= 0:
            continue
        pc = VectorClock()
        pc.require_at_least(p, tick)
        w = nc.sync.nop(nofuse=True)
        wait_clock.add_sem_waits(w.ins, ScopedClock({None: pc}))
    nc.sync.drain()
    nc.all_engine_barrier()
    assert self.sems is not None
    popped = nc._tile_sem_poison_stack.pop()
    assert popped is self._sem_poison
    if not _SKIP_SEM_CLEAR[0]:
        # ~9us of serial EVENT_SEMAPHORE tail; the runtime re-zeroes
        # semaphores per execution, so the in-NEFF clear is redundant
        # (verified: repeat executions of the same NEFF stay correct)
        nc.clear_and_free_semaphores(list(self.sems.allocated().values()))
        nc.all_engine_barrier()


_tile.TileContext._drain_and_barrier = _split_drain_and_barrier

_wsplit_counter = [0]


def _legalize_single_wait(nc):
    """Hoist extra sem waits onto preceding same-engine NoOps.

    This walrus build encodes at most ONE sync-wait command per TPB
    instruction; Tile's sem-assignment pass freely attaches several.
    Splitting extras onto immediately-preceding NoOps on the same engine
    preserves program order (engines issue in order), hence semantics."""
    import bass_rust

    for fn in nc.m.functions:
        for blk in fn.blocks:
            insts = list(blk.instructions)
            if not any(
                ins.sync_info is not None and len(ins.sync_info.on_wait) > 1
                for ins in insts
            ):
                continue
            out = []
            for ins in insts:
                si = ins.sync_info
                waits = list(si.on_wait) if si is not None else []
                if len(waits) > 1:
                    for w in waits[:-1]:
                        _wsplit_counter[0] += 1
                        nop = mybir.InstNoOp(
                            name=f"I-wsplit-{_wsplit_counter[0]}", ins=[], outs=[]
                        )
                        nop.engine = ins.engine
                        nop.sync_info = bass_rust.SyncInfo(
                            on_wait=[w], on_update=[]
                        )
                        out.append(nop)
                    si.on_wait = [waits[-1]]
                out.append(ins)
            blk.instructions = out


# --------------------------------------------------------------------------
# Problem constants (hardcoded per contract: kernel.py is self-contained).
# --------------------------------------------------------------------------
B, F, D = 4096, 32, 64
NCORES = 8
BL = B // NCORES          # 512 batch rows per core
PT = 128                  # batch tile = SBUF partition count
TILES = BL // PT          # 4 tiles per core
NPAIR = F * (F - 1) // 2  # 496
# pair index of (i, i+1) within itertools.combinations(range(F), 2) order
IDX0 = [0] * F
for _i in range(1, F):
    IDX0[_i] = IDX0[_i - 1] + (F - _i)
# per-parity column offset of field i's run inside its wt half
POFF = [0] * F
for _i in range(2, F):
    POFF[_i] = POFF[_i - 2] + (F - 1 - (_i - 2)) * D
WT_COLS = max(POFF[30] + 1 * D, POFF[31])  # even half is the larger: 16384
WT_COLS = max(WT_COLS, 16384)

F32 = mybir.dt.float32

_nc_cache = {}


CH = 512            # PSUM group = one matmul = one bank
OUT_W = 8192        # max cols per aggregated output tile
FLUSH = 2048        # min cols per rolling output DMA
RUN_W = 1984        # max cols of a field run (i=0)


def _plan_paths(fracs):
    """Assign each field run i (cols (31-i)*64) to an epilogue path.

    'a': DVE multiplies straight out of PSUM per CH-chunk (1x PSUM-read
         mode, ~110 G elem/s).
    'f': ACT copies each chunk PSUM->SBUF bf16 (~123 G elem/s), one DVE
         SBUF x SBUF bf16 multiply per run (2x packed mode).
    'g': same ACT copies, GpSimd does the run multiply (~85 G elem/s).
    PSUM is drained only by DVE('a') + ACT('f'/'g'); fractions balance
    the three multiply engines against those measured rates.
    """
    # measured G elem/s per engine-path; assign runs (largest first) to
    # minimize the projected makespan over {DVE, ACT, GpSimd, shared-port}.
    # 'shared' models the exclusive SBUF port pair that DVE packed-mode
    # ('f') ops and ALL GpSimd ops arbitrate for — they serialize, so an
    # f-mul and a g-mul can never overlap.
    # f (DVE packed-mode) is disabled by default: with GpSimd busy most of
    # the span, every f-mul stalls on the shared-port lock and the wait
    # counts as DVE busy time — a+g is contention-free (DVE 1x uses its
    # dedicated ports; GpSimd alone owns the shared pair). The g set is
    # the k largest runs (a prefix, since run size is monotonic in i)
    # with k chosen to balance DVE against GpSimd.
    # Three epilogue paths, assigned per run by local-makespan greedy in
    # i order (balances totals AND interleaves in time):
    #   'a': DVE multiplies straight out of f32 PSUM (1x, ~106 G/s)
    #   'g': ACT copies chunks -> SBUF bf16, one GpSimd run-mul (~63 G/s
    #        plus ~0.35us fixed dispatch per run-mul)
    # (an 'h' path via bf16 PSUM is impossible: only Matmult/Memset may
    # write bf16 to PSUM, so ACT can't stage packed operands there)
    # The g "planning" rate is biased above GpSimd's real ~63 G/s: the
    # least-loaded walk quantizes by whole runs and systematically
    # leaves GpSimd ~20% short at the true rate; 76 lands the real
    # busies at DVE ~99us / GP ~92us (vs 105/82 unbiased).
    rates = fracs or {"a": 106.0, "act": 104.0, "g": 76.0}
    acc = {"dve": 0.0, "act": 0.0, "gp": 0.0}
    path = {}
    for i in range(F - 1):
        cols = (F - 1 - i) * D
        e = cols * PT / 1e3
        if cols >= 768 and rates.get("g", 0) > 0 and acc["gp"] <= acc["dve"]:
            path[i] = "g"
            acc["gp"] += e / rates["g"]
            acc["act"] += e / rates["act"]
        else:
            path[i] = "a"
            acc["dve"] += e / rates["a"]
    path["_busy"] = acc
    return path


def _plan_out_chunks():
    """Group consecutive m's (field pairs 2m, 2m+1) into output chunks of
    <= OUT_W contiguous y columns; each chunk is one SBUF tile + one DMA."""
    chunks = []
    cur_ms, cur_cols, cbase = [], 0, 0
    for m in range(F // 2):
        mc = sum((F - 1 - i) * D for i in (2 * m, 2 * m + 1) if i <= F - 2)
        if cur_ms and cur_cols + mc > OUT_W:
            chunks.append((cur_ms, cbase, cur_cols))
            cbase += cur_cols
            cur_ms, cur_cols = [], 0
        cur_ms.append(m)
        cur_cols += mc
    chunks.append((cur_ms, cbase, cur_cols))
    return chunks


def _build_bass(mm_dt=F32, psum_cols=CH, psum_bufs=4, io_bufs=3, out_bufs=4,
                derive_x=False, x_dt=F32, out_dt=F32,
                fracs=None, cp_bufs=3, pe_tile=True):
    nc = bass.Bass(trn_type="TRN2")
    x_d = nc.dram_tensor("x", [BL, F * D], x_dt, kind="ExternalInput")
    xt_d = nc.dram_tensor("xt", [PT, TILES * (F // 2) * PT], mm_dt,
                          kind="ExternalInput")
    wt_d = nc.dram_tensor("wt", [PT, WT_COLS], mm_dt, kind="ExternalInput")
    y_d = nc.dram_tensor("y", [BL, NPAIR * D], out_dt, kind="ExternalOutput")

    CB = (F // 2) * PT  # 2048 xt cols per batch tile
    run_path = _plan_paths(fracs)
    out_chunks = _plan_out_chunks()

    with TileContext(nc) as tc:
        with (
            tc.tile_pool(name="wtp", bufs=1) as wtp,
            tc.tile_pool(name="iop", bufs=io_bufs) as iop,
            tc.tile_pool(name="outp", bufs=out_bufs) as outp,
            tc.tile_pool(name="cpp", bufs=cp_bufs) as cpp,
            tc.tile_pool(name="pp", bufs=psum_bufs, space="PSUM") as pp,
        ):
            wt_s = wtp.tile([PT, WT_COLS], mm_dt)
            xs_t, xts_t = [None] * TILES, [None] * TILES

            def load_tile(t, first=False):
                # xt first: the first matmul needs it, x only at the
                # first multiply
                xt_s = iop.tile([PT, CB], mm_dt, tag="xt", name=f"xt_{t}")
                nc.sync.dma_start(
                    out=xt_s, in_=xt_d[:, t * CB : (t + 1) * CB]
                )
                if first:
                    # squeeze wt chunk 0 in before x so the first matmul
                    # (xt + wt c0) is unblocked one transfer earlier
                    nc.sync.dma_start(
                        out=wt_s[:, 0:WCH], in_=wt_d[:, 0:WCH]
                    )
                x_s = iop.tile([PT, F * D], x_dt, tag="x", name=f"x_{t}")
                nc.sync.dma_start(out=x_s, in_=x_d[t * PT : (t + 1) * PT, :])
                xs_t[t], xts_t[t] = x_s, xt_s

            WCH = 2048
            load_tile(0, first=True)
            for w0 in range(WCH, WT_COLS, WCH):
                nc.sync.dma_start(
                    out=wt_s[:, w0 : w0 + WCH], in_=wt_d[:, w0 : w0 + WCH]
                )
            for t in range(TILES):
                if t + 1 < TILES:
                    load_tile(t + 1)  # prefetch (io_bufs >= 3 keeps the
                    # sync queue from blocking on ring reuse)
                x_s, xt_s = xs_t[t], xts_t[t]
                for ms, cbase, ccols in out_chunks:
                    out_s = outp.tile(
                        [PT, OUT_W], out_dt, tag="o", name=f"o_{t}_{cbase}"
                    )
                    flushed = 0  # cols of out_s already sent to HBM
                    done = 0     # cols fully written by the mul engines
                    for m in ms:
                        # all chunks of one (m, parity) run are emitted
                        # back-to-back: they share one stationary tile, so
                        # the compiler can skip redundant LDWEIGHTS; the
                        # two parities still alternate at run granularity
                        per_par, runs = [], []
                        for par in (0, 1):
                            i = 2 * m + par
                            if i > F - 2:
                                continue
                            ncol = (F - 1 - i) * D
                            ch = [
                                (par, i, g0, min(CH, ncol - g0))
                                for g0 in range(0, ncol, CH)
                            ]
                            per_par.append(ch)
                            runs.append((par, i, ncol))
                        cps = {}
                        for par, i, ncol in runs:
                            if run_path[i] == "g":
                                cps[i] = cpp.tile(
                                    [PT, RUN_W], out_dt,
                                    tag=f"cg{par}",
                                    name=f"c_{t}_{i}",
                                )
                        for ch in per_par:
                            par = ch[0][0]
                            lhsT = xt_s[par * D : (par + 1) * D,
                                        m * PT : (m + 1) * PT]
                            for par, i, g0, gcols in ch:
                                ps = pp.tile(
                                    [PT, CH], F32, tag=f"ps{par}",
                                    name=f"ps_{t}_{i}_{g0}",
                                )
                                nc.tensor.matmul(
                                    ps[:, :gcols],
                                    lhsT,
                                    wt_s[par * D : (par + 1) * D,
                                         POFF[i] + g0 : POFF[i] + g0 + gcols],
                                    start=True,
                                    stop=True,
                                    tile_position=(par * D, 0),
                                )
                                kind = run_path[i]
                                if kind == "g":
                                    nc.scalar.copy(
                                        out=cps[i][:, g0 : g0 + gcols],
                                        in_=ps[:, :gcols],
                                    )
                                    continue
                                oc = IDX0[i] * D + g0 - cbase
                                nc.vector.tensor_mul(
                                    out=out_s[:, oc : oc + gcols],
                                    in0=ps[:, :gcols],
                                    in1=x_s[:, (i + 1) * D + g0
                                            : (i + 1) * D + g0 + gcols],
                                )
                        for par, i, ncol in runs:
                            if run_path[i] != "g":
                                continue
                            oc = IDX0[i] * D - cbase
                            nc.gpsimd.tensor_mul(
                                out=out_s[:, oc : oc + ncol],
                                in0=cps[i][:, :ncol],
                                in1=x_s[:, (i + 1) * D : (i + 1) * D + ncol],
                            )
                        # rolling flush: ship finished column ranges while
                        # later m's are still computing, so output DMA
                        # overlaps compute at sub-chunk granularity
                        done += sum(nc_ for _, _, nc_ in runs)
                        if done - flushed >= FLUSH:
                            nc.sync.dma_start(
                                out=y_d[t * PT : (t + 1) * PT,
                                        cbase + flushed : cbase + done],
                                in_=out_s[:, flushed : done],
                            )
                            flushed = done
                    if done > flushed:
                        nc.sync.dma_start(
                            out=y_d[t * PT : (t + 1) * PT,
                                    cbase + flushed : cbase + done],
                            in_=out_s[:, flushed : done],
                        )
    _legalize_single_wait(nc)
    return nc


def _get_nc(mm_dt, psum_cols, psum_bufs, io_bufs=3, out_bufs=3, derive_x=False,
            x_dt=F32, out_dt=F32, fracs=None, cp_bufs=2, pe_tile=True):
    key = (str(mm_dt), psum_cols, psum_bufs, io_bufs, out_bufs, derive_x,
           str(x_dt), str(out_dt), str(sorted(fracs.items())) if fracs else "",
           cp_bufs, pe_tile)
    if key not in _nc_cache:
        _nc_cache[key] = _build_bass(
            mm_dt, psum_cols, psum_bufs, io_bufs, out_bufs, derive_x,
            x_dt, out_dt, fracs, cp_bufs, pe_tile
        )
    return _nc_cache[key]


def _prep_inputs(x, W, derive_x=False, mm_dt=F32, x_dt=F32):
    mm_np = mybir.dt.np(mm_dt)
    x_np = mybir.dt.np(x_dt)
    x = np.ascontiguousarray(np.asarray(x, dtype=np.float32))
    W = np.ascontiguousarray(np.asarray(W, dtype=np.float32))
    # wt2[par*64+d, POFF[i] + (j-i-1)*64 + o] = W[(i,j), o, d]
    wt2 = np.zeros((PT, WT_COLS), dtype=np.float32)
    for i in range(F - 1):
        par = i % 2
        npair = F - 1 - i
        blk = W[IDX0[i] : IDX0[i] + npair]           # [npair, D, D]
        blk = blk.transpose(2, 0, 1).reshape(D, npair * D)
        wt2[par * D : (par + 1) * D, POFF[i] : POFF[i] + npair * D] = blk
    wt2 = np.ascontiguousarray(wt2.astype(mm_np))
    in_maps = []
    for c in range(NCORES):
        xl = x[c * BL : (c + 1) * BL]                      # [512, 32, 64]
        x_in = np.ascontiguousarray(xl.reshape(BL, F * D).astype(x_np))
        # xt2[par*64+d, t*2048 + m*128 + b] = xl[t*128+b, 2m+par, d]
        xt2 = np.ascontiguousarray(
            xl.reshape(TILES, PT, F // 2, 2, D).transpose(3, 4, 0, 2, 1)
            .astype(mm_np)
        ).reshape(PT, TILES * (F // 2) * PT)
        m = {"xt": xt2, "wt": wt2}
        if derive_x:
            m["ident"] = np.eye(PT, dtype=mm_np)
        else:
            m["x"] = x_in
        in_maps.append(m)
    return in_maps


def _run(x, W, trace=False, mm_dt=None, psum_cols=CH, psum_bufs=4, io_bufs=3,
         out_bufs=4, derive_x=False, x_dt=None, out_dt=None, fracs=None,
         cp_bufs=3, pe_tile=True):
    # Default: all-bf16 I/O. The kernel is HBM-bound, so halving the bytes
    # of every stream (xt/x/wt reads, y write) halves the roofline time.
    # Measured accuracy of the bf16 pipeline vs the fp32 reference:
    # ~7e-3 scale-relative max error (gate is 2e-2). Pass
    # mm_dt=mybir.dt.float32r, x_dt=out_dt=F32 for the old f32 pipeline.
    if mm_dt is None:
        mm_dt = mybir.dt.bfloat16
    if x_dt is None:
        x_dt = mybir.dt.bfloat16
    if out_dt is None:
        out_dt = mybir.dt.bfloat16
    nc = _get_nc(mm_dt, psum_cols, psum_bufs, io_bufs, out_bufs, derive_x,
                 x_dt, out_dt, fracs, cp_bufs, pe_tile)
    in_maps = _prep_inputs(x, W, derive_x, mm_dt, x_dt)
    res = run_bass_kernel_spmd(nc, in_maps, core_ids=list(range(NCORES)), trace=trace)
    y = np.concatenate(
        [res.results[c]["y"].reshape(BL, NPAIR, D).astype(np.float32)
         for c in range(NCORES)],
        axis=0,
    )
    return y, res


def kernel(x, W):
    y, _ = _run(x, W)
    return y



# revision 62
# speedup vs baseline: 1.1694x; 1.1694x over previous
"""Trainium2 Bass kernel for BiLinearInteractionLayer.

Computes, for every field pair p=(i,j), i<j, of F=32 fields:
    y[b, p, :] = (x[b, i, :] @ W[p].T) * x[b, j, :]
x: [4096, 32, 64] f32, W: [496, 64, 64] f32 -> y: [4096, 496, 64] f32.

Sharding: data-parallel over the batch dim across 8 NeuronCores (512
rows each); the weight stack is replicated.

The kernel is HBM-bound (the 520 MB output write dominates), so all
I/O runs in bf16: inputs are rounded on the host, the output is
upcast to f32 after the gather. Measured scale-relative max error of
the bf16 pipeline is ~7e-3 (harness gate 2e-2).

Per-core algorithm (batch tile of 128 rows at a time):
  - Host pre-transposes layouts (free): the contraction dim d lands on
    SBUF partitions with clean contiguous DMAs, no on-device transposes.
  - For each first-field i, the pairs (i, i+1..31) form a contiguous
    run in both the pair axis and the transposed weight columns: one
    stationary xT_i [64d, 128b] serves bf16 matmuls streaming 512-col
    chunks of W^T into single-bank PSUM tiles (ring of 4 per parity).
    Even fields sit on PE row group 0, odd on row group 2; runs of the
    two parities alternate so stationary loads overlap the other
    stream's matmuls.
  - Epilogue (the y = proj * xj multiply) is split across engines by a
    rate-balanced per-run plan: DVE multiplies most runs straight out
    of PSUM (1x PSUM-read mode, ~106 G elem/s); for the rest, ACT
    copies each chunk PSUM->SBUF bf16 (~104 G elem/s) and GpSimd does
    one multiply per run (~63 G elem/s). DVE-1x and GpSimd use
    disjoint SBUF ports, so the three engines run concurrently (DVE
    packed 2x modes are avoided: they serialize against GpSimd on the
    shared SBUF port pair).
  - Outputs accumulate in large SBUF tiles covering up to 8192
    contiguous y columns and are flushed to HBM in rolling >=2048-col
    DMAs (16 KB row segments), overlapping compute at sub-chunk
    granularity.
  - Tile 0's x/xt DMAs are issued before the weight stack and each
    tile prefetches the next tile's inputs, so compute starts ~3 us in
    and never stalls on input loads.
"""

import itertools

import numpy as np

import concourse.bass as bass
import concourse.mybir as mybir
import concourse.tile as _tile
from concourse.bass_utils import run_bass_kernel_spmd
from concourse.tile import TileContext
from concourse.tile_scheduler import N_PROCS
from concourse.vector_clock import ScopedClock, VectorClock

# --------------------------------------------------------------------------
# Tail-drain patch: the staged walrus rejects >1 sync-wait command on a
# TPB_CTRL (Drain) instruction, but the stock Tile tail-drain attaches one
# wait per outstanding sem lane to a single Drain. Replace it with a ladder
# of single-wait SP nops (one per proc lane) followed by a wait-less drain.
# --------------------------------------------------------------------------


# Skipping the end-of-NEFF semaphore clear (~9us serial tail) HANGS the
# second execution of the same NEFF — the runtime does NOT re-zero
# semaphores between executions. Must stay False.
_SKIP_SEM_CLEAR = [False]


def _split_drain_and_barrier(self, tick_clock, wait_clock):
    nc = self.nc
    g = tick_clock.global_clock
    for p in range(N_PROCS):
        tick = g.peek_next(p) - 1
        if tick <= 0:
            continue
        pc = VectorClock()
        pc.require_at_least(p, tick)
        w = nc.sync.nop(nofuse=True)
        wait_clock.add_sem_waits(w.ins, ScopedClock({None: pc}))
    nc.sync.drain()
    nc.all_engine_barrier()
    assert self.sems is not None
    popped = nc._tile_sem_poison_stack.pop()
    assert popped is self._sem_poison
    if not _SKIP_SEM_CLEAR[0]:
        # ~9us of serial EVENT_SEMAPHORE tail; the runtime re-zeroes
        # semaphores per execution, so the in-NEFF clear is redundant
        # (verified: repeat executions of the same NEFF stay correct)
        nc.clear_and_free_semaphores(list(self.sems.allocated().values()))
        nc.all_engine_barrier()


_tile.TileContext._drain_and_barrier = _split_drain_and_barrier

_wsplit_counter = [0]


def _legalize_single_wait(nc):
    """Hoist extra sem waits onto preceding same-engine NoOps.

    This walrus build encodes at most ONE sync-wait command per TPB
    instruction; Tile's sem-assignment pass freely attaches several.
    Splitting extras onto immediately-preceding NoOps on the same engine
    preserves program order (engines issue in order), hence semantics."""
    import bass_rust

    for fn in nc.m.functions:
        for blk in fn.blocks:
            insts = list(blk.instructions)
            if not any(
                ins.sync_info is not None and len(ins.sync_info.on_wait) > 1
                for ins in insts
            ):
                continue
            out = []
            for ins in insts:
                si = ins.sync_info
                waits = list(si.on_wait) if si is not None else []
                if len(waits) > 1:
                    for w in waits[:-1]:
                        _wsplit_counter[0] += 1
                        nop = mybir.InstNoOp(
                            name=f"I-wsplit-{_wsplit_counter[0]}", ins=[], outs=[]
                        )
                        nop.engine = ins.engine
                        nop.sync_info = bass_rust.SyncInfo(
                            on_wait=[w], on_update=[]
                        )
                        out.append(nop)
                    si.on_wait = [waits[-1]]
                out.append(ins)
            blk.instructions = out


# --------------------------------------------------------------------------
# Problem constants (hardcoded per contract: kernel.py is self-contained).
# --------------------------------------------------------------------------
B, F, D = 4096, 32, 64
NCORES = 8
BL = B // NCORES          # 512 batch rows per core
PT = 128                  # batch tile = SBUF partition count
TILES = BL // PT          # 4 tiles per core
NPAIR = F * (F - 1) // 2  # 496
# pair index of (i, i+1) within itertools.combinations(range(F), 2) order
IDX0 = [0] * F
for _i in range(1, F):
    IDX0[_i] = IDX0[_i - 1] + (F - _i)
# per-parity column offset of field i's run inside its wt half
POFF = [0] * F
for _i in range(2, F):
    POFF[_i] = POFF[_i - 2] + (F - 1 - (_i - 2)) * D
WT_COLS = max(POFF[30] + 1 * D, POFF[31])  # even half is the larger: 16384
WT_COLS = max(WT_COLS, 16384)

F32 = mybir.dt.float32

_nc_cache = {}


CH = 512            # PSUM group = one matmul = one bank
OUT_W = 8192        # max cols per aggregated output tile
FLUSH = 2048        # min cols per rolling output DMA
RUN_W = 1984        # max cols of a field run (i=0)


def _plan_paths(fracs):
    """Assign each field run i (cols (31-i)*64) to an epilogue path.

    'a': DVE multiplies straight out of PSUM per CH-chunk (1x PSUM-read
         mode, ~110 G elem/s).
    'f': ACT copies each chunk PSUM->SBUF bf16 (~123 G elem/s), one DVE
         SBUF x SBUF bf16 multiply per run (2x packed mode).
    'g': same ACT copies, GpSimd does the run multiply (~85 G elem/s).
    PSUM is drained only by DVE('a') + ACT('f'/'g'); fractions balance
    the three multiply engines against those measured rates.
    """
    # measured G elem/s per engine-path; assign runs (largest first) to
    # minimize the projected makespan over {DVE, ACT, GpSimd, shared-port}.
    # 'shared' models the exclusive SBUF port pair that DVE packed-mode
    # ('f') ops and ALL GpSimd ops arbitrate for — they serialize, so an
    # f-mul and a g-mul can never overlap.
    # f (DVE packed-mode) is disabled by default: with GpSimd busy most of
    # the span, every f-mul stalls on the shared-port lock and the wait
    # counts as DVE busy time — a+g is contention-free (DVE 1x uses its
    # dedicated ports; GpSimd alone owns the shared pair). The g set is
    # the k largest runs (a prefix, since run size is monotonic in i)
    # with k chosen to balance DVE against GpSimd.
    # Three epilogue paths, assigned per run by local-makespan greedy in
    # i order (balances totals AND interleaves in time):
    #   'a': DVE multiplies straight out of f32 PSUM (1x, ~106 G/s)
    #   'g': ACT copies chunks -> SBUF bf16, one GpSimd run-mul (~63 G/s
    #        plus ~0.35us fixed dispatch per run-mul)
    # (an 'h' path via bf16 PSUM is impossible: only Matmult/Memset may
    # write bf16 to PSUM, so ACT can't stage packed operands there)
    # The g "planning" rate is biased above GpSimd's real ~63 G/s: the
    # least-loaded walk quantizes by whole runs and systematically
    # leaves GpSimd ~20% short at the true rate; 76 lands the real
    # busies at DVE ~99us / GP ~92us (vs 105/82 unbiased).
    rates = fracs or {"a": 106.0, "act": 104.0, "g": 76.0}
    acc = {"dve": 0.0, "act": 0.0, "gp": 0.0}
    path = {}
    for i in range(F - 1):
        cols = (F - 1 - i) * D
        e = cols * PT / 1e3
        if cols >= 768 and rates.get("g", 0) > 0 and acc["gp"] <= acc["dve"]:
            path[i] = "g"
            acc["gp"] += e / rates["g"]
            acc["act"] += e / rates["act"]
        else:
            path[i] = "a"
            acc["dve"] += e / rates["a"]
    path["_busy"] = acc
    return path


def _plan_out_chunks():
    """Group consecutive m's (field pairs 2m, 2m+1) into output chunks of
    <= OUT_W contiguous y columns; each chunk is one SBUF tile + one DMA."""
    chunks = []
    cur_ms, cur_cols, cbase = [], 0, 0
    for m in range(F // 2):
        mc = sum((F - 1 - i) * D for i in (2 * m, 2 * m + 1) if i <= F - 2)
        if cur_ms and cur_cols + mc > OUT_W:
            chunks.append((cur_ms, cbase, cur_cols))
            cbase += cur_cols
            cur_ms, cur_cols = [], 0
        cur_ms.append(m)
        cur_cols += mc
    chunks.append((cur_ms, cbase, cur_cols))
    return chunks


def _build_bass(mm_dt=F32, psum_cols=CH, psum_bufs=4, io_bufs=3, out_bufs=4,
                derive_x=False, x_dt=F32, out_dt=F32,
                fracs=None, cp_bufs=3, pe_tile=True):
    nc = bass.Bass(trn_type="TRN2")
    x_d = nc.dram_tensor("x", [BL, F * D], x_dt, kind="ExternalInput")
    xt_d = nc.dram_tensor("xt", [PT, TILES * (F // 2) * PT], mm_dt,
                          kind="ExternalInput")
    wt_d = nc.dram_tensor("wt", [PT, WT_COLS], mm_dt, kind="ExternalInput")
    y_d = nc.dram_tensor("y", [BL, NPAIR * D], out_dt, kind="ExternalOutput")

    CB = (F // 2) * PT  # 2048 xt cols per batch tile
    run_path = _plan_paths(fracs)
    out_chunks = _plan_out_chunks()

    with TileContext(nc) as tc:
        with (
            tc.tile_pool(name="wtp", bufs=1) as wtp,
            tc.tile_pool(name="iop", bufs=io_bufs) as iop,
            tc.tile_pool(name="outp", bufs=out_bufs) as outp,
            tc.tile_pool(name="cpp", bufs=cp_bufs) as cpp,
            tc.tile_pool(name="pp", bufs=psum_bufs, space="PSUM") as pp,
        ):
            wt_s = wtp.tile([PT, WT_COLS], mm_dt)
            xs_t, xts_t = [None] * TILES, [None] * TILES

            def load_tile(t):
                # xt first: the first matmul needs it, x only at the
                # first multiply. (Interleaving wt chunk 0 between them
                # measured 16us SLOWER — keep wt strictly after x/xt.)
                xt_s = iop.tile([PT, CB], mm_dt, tag="xt", name=f"xt_{t}")
                nc.sync.dma_start(
                    out=xt_s, in_=xt_d[:, t * CB : (t + 1) * CB]
                )
                x_s = iop.tile([PT, F * D], x_dt, tag="x", name=f"x_{t}")
                nc.sync.dma_start(out=x_s, in_=x_d[t * PT : (t + 1) * PT, :])
                xs_t[t], xts_t[t] = x_s, xt_s

            # tile 0's inputs land before the weight stack so the first
            # matmul can start as soon as wt chunk 0 arrives
            load_tile(0)
            WCH = 2048
            for w0 in range(0, WT_COLS, WCH):
                nc.sync.dma_start(
                    out=wt_s[:, w0 : w0 + WCH], in_=wt_d[:, w0 : w0 + WCH]
                )
            for t in range(TILES):
                if t + 1 < TILES:
                    load_tile(t + 1)  # prefetch (io_bufs >= 3 keeps the
                    # sync queue from blocking on ring reuse)
                x_s, xt_s = xs_t[t], xts_t[t]
                for ms, cbase, ccols in out_chunks:
                    out_s = outp.tile(
                        [PT, OUT_W], out_dt, tag="o", name=f"o_{t}_{cbase}"
                    )
                    flushed = 0  # cols of out_s already sent to HBM
                    done = 0     # cols fully written by the mul engines
                    for m in ms:
                        # all chunks of one (m, parity) run are emitted
                        # back-to-back: they share one stationary tile, so
                        # the compiler can skip redundant LDWEIGHTS; the
                        # two parities still alternate at run granularity
                        per_par, runs = [], []
                        for par in (0, 1):
                            i = 2 * m + par
                            if i > F - 2:
                                continue
                            ncol = (F - 1 - i) * D
                            ch = [
                                (par, i, g0, min(CH, ncol - g0))
                                for g0 in range(0, ncol, CH)
                            ]
                            per_par.append(ch)
                            runs.append((par, i, ncol))
                        cps = {}
                        for par, i, ncol in runs:
                            if run_path[i] == "g":
                                cps[i] = cpp.tile(
                                    [PT, RUN_W], out_dt,
                                    tag=f"cg{par}",
                                    name=f"c_{t}_{i}",
                                )
                        for ch in per_par:
                            par = ch[0][0]
                            lhsT = xt_s[par * D : (par + 1) * D,
                                        m * PT : (m + 1) * PT]
                            for par, i, g0, gcols in ch:
                                ps = pp.tile(
                                    [PT, CH], F32, tag=f"ps{par}",
                                    name=f"ps_{t}_{i}_{g0}",
                                )
                                nc.tensor.matmul(
                                    ps[:, :gcols],
                                    lhsT,
                                    wt_s[par * D : (par + 1) * D,
                                         POFF[i] + g0 : POFF[i] + g0 + gcols],
                                    start=True,
                                    stop=True,
                                    tile_position=(par * D, 0),
                                )
                                kind = run_path[i]
                                if kind == "g":
                                    nc.scalar.copy(
                                        out=cps[i][:, g0 : g0 + gcols],
                                        in_=ps[:, :gcols],
                                    )
                                    continue
                                oc = IDX0[i] * D + g0 - cbase
                                nc.vector.tensor_mul(
                                    out=out_s[:, oc : oc + gcols],
                                    in0=ps[:, :gcols],
                                    in1=x_s[:, (i + 1) * D + g0
                                            : (i + 1) * D + g0 + gcols],
                                )
                        for par, i, ncol in runs:
                            if run_path[i] != "g":
                                continue
                            oc = IDX0[i] * D - cbase
                            nc.gpsimd.tensor_mul(
                                out=out_s[:, oc : oc + ncol],
                                in0=cps[i][:, :ncol],
                                in1=x_s[:, (i + 1) * D : (i + 1) * D + ncol],
                            )
                        # rolling flush: ship finished column ranges while
                        # later m's are still computing, so output DMA
                        # overlaps compute at sub-chunk granularity
                        done += sum(nc_ for _, _, nc_ in runs)
                        if done - flushed >= FLUSH:
                            nc.sync.dma_start(
                                out=y_d[t * PT : (t + 1) * PT,
                                        cbase + flushed : cbase + done],
                                in_=out_s[:, flushed : done],
                            )
                            flushed = done
                    if done > flushed:
                        nc.sync.dma_start(
                            out=y_d[t * PT : (t + 1) * PT,
                                    cbase + flushed : cbase + done],
                            in_=out_s[:, flushed : done],
                        )
    _legalize_single_wait(nc)
    return nc


def _get_nc(mm_dt, psum_cols, psum_bufs, io_bufs=3, out_bufs=3, derive_x=False,
            x_dt=F32, out_dt=F32, fracs=None, cp_bufs=2, pe_tile=True):
    key = (str(mm_dt), psum_cols, psum_bufs, io_bufs, out_bufs, derive_x,
           str(x_dt), str(out_dt), str(sorted(fracs.items())) if fracs else "",
           cp_bufs, pe_tile)
    if key not in _nc_cache:
        _nc_cache[key] = _build_bass(
            mm_dt, psum_cols, psum_bufs, io_bufs, out_bufs, derive_x,
            x_dt, out_dt, fracs, cp_bufs, pe_tile
        )
    return _nc_cache[key]


def _prep_inputs(x, W, derive_x=False, mm_dt=F32, x_dt=F32):
    mm_np = mybir.dt.np(mm_dt)
    x_np = mybir.dt.np(x_dt)
    x = np.ascontiguousarray(np.asarray(x, dtype=np.float32))
    W = np.ascontiguousarray(np.asarray(W, dtype=np.float32))
    # wt2[par*64+d, POFF[i] + (j-i-1)*64 + o] = W[(i,j), o, d]
    wt2 = np.zeros((PT, WT_COLS), dtype=np.float32)
    for i in range(F - 1):
        par = i % 2
        npair = F - 1 - i
        blk = W[IDX0[i] : IDX0[i] + npair]           # [npair, D, D]
        blk = blk.transpose(2, 0, 1).reshape(D, npair * D)
        wt2[par * D : (par + 1) * D, POFF[i] : POFF[i] + npair * D] = blk
    wt2 = np.ascontiguousarray(wt2.astype(mm_np))
    in_maps = []
    for c in range(NCORES):
        xl = x[c * BL : (c + 1) * BL]                      # [512, 32, 64]
        x_in = np.ascontiguousarray(xl.reshape(BL, F * D).astype(x_np))
        # xt2[par*64+d, t*2048 + m*128 + b] = xl[t*128+b, 2m+par, d]
        xt2 = np.ascontiguousarray(
            xl.reshape(TILES, PT, F // 2, 2, D).transpose(3, 4, 0, 2, 1)
            .astype(mm_np)
        ).reshape(PT, TILES * (F // 2) * PT)
        m = {"xt": xt2, "wt": wt2}
        if derive_x:
            m["ident"] = np.eye(PT, dtype=mm_np)
        else:
            m["x"] = x_in
        in_maps.append(m)
    return in_maps


def _run(x, W, trace=False, mm_dt=None, psum_cols=CH, psum_bufs=4, io_bufs=3,
         out_bufs=4, derive_x=False, x_dt=None, out_dt=None, fracs=None,
         cp_bufs=3, pe_tile=True):
    # Default: all-bf16 I/O. The kernel is HBM-bound, so halving the bytes
    # of every stream (xt/x/wt reads, y write) halves the roofline time.
    # Measured accuracy of the bf16 pipeline vs the fp32 reference:
    # ~7e-3 scale-relative max error (gate is 2e-2). Pass
    # mm_dt=mybir.dt.float32r, x_dt=out_dt=F32 for the old f32 pipeline.
    if mm_dt is None:
        mm_dt = mybir.dt.bfloat16
    if x_dt is None:
        x_dt = mybir.dt.bfloat16
    if out_dt is None:
        out_dt = mybir.dt.bfloat16
    nc = _get_nc(mm_dt, psum_cols, psum_bufs, io_bufs, out_bufs, derive_x,
                 x_dt, out_dt, fracs, cp_bufs, pe_tile)
    in_maps = _prep_inputs(x, W, derive_x, mm_dt, x_dt)
    res = run_bass_kernel_spmd(nc, in_maps, core_ids=list(range(NCORES)), trace=trace)
    y = np.concatenate(
        [res.results[c]["y"].reshape(BL, NPAIR, D).astype(np.float32)
         for c in range(NCORES)],
        axis=0,
    )
    return y, res


def kernel(x, W):
    y, _ = _run(x, W)
    return y

